# revision 7
# baseline (speedup 1.0000x reference)
"""CNP loss kernel for 8 Trainium2 NeuronCores.

Math (reference):
  h  = relu(O @ hW0 + hb0); h = relu(h @ hW1 + hb1); h = h @ hW2 + hb2
  r  = mean(h, axis=0)                               # [128]
  xr = [broadcast(r), T[:, :1]]                      # [Nt, 129]
  g  = 4x relu layers + linear -> phi [Nt, 2]
  mu = phi[:, 0]; sig = softplus(phi[:, 1]); y = T[:, 1]
  logp = mean(-0.5 z^2 - ln sig - 0.5 ln 2pi), z = (y - mu)/sig

Exact algebraic restructurings used:
  * mean-pool folds through the last (linear) h-layer: only the 32-wide
    column sum of relu-layer-1 activations needs the cross-core AllReduce.
  * g-layer 0 splits: xr @ gW0 = r @ gW0[:128] + t * gW0[128]; the shared
    part c0 = r @ gW0[:128] + gb0 becomes a per-partition bias, the rest a
    K=1 outer product.
  * zero-padded O rows add a constant relu(relu(hb0)@hW1+hb1) per row to the
    pooled sum; corrected via an adjusted hb2 bias computed on the host.
  * the final mean over targets finishes on the host from per-core partial
    sums (no second collective).

Layouts: activations stay feature-major ([features, rows]); the tiny layers
are packed to full 128-partition density with block-diagonal stationary
operands and tile_position row/col tiling.
"""

import numpy as np

import concourse.bass as bass
import concourse.bacc as bacc
import concourse.tile as tile
from concourse import mybir
from concourse.bass_utils import run_bass_kernel_spmd

FP = mybir.dt.float32
AF = mybir.ActivationFunctionType
ALU = mybir.AluOpType

N_CORES = 8
NCTX, NTGT = 100000, 200000
OC = NCTX // N_CORES            # 12500 ctx rows per core
TC = NTGT // N_CORES            # 25000 tgt rows per core
F = 500                         # rows per chunk (matmul free dim)
H_CHUNKS = OC // F              # 25 -> padded to 32 (2 passes of 16)
H_PASSES = 2
G_CHUNKS = TC // F              # 50 -> padded to 64 (16 passes of 4)
G_PASSES = 16
N_PAD_H = (16 * H_PASSES - H_CHUNKS) * F * N_CORES  # 28000 padded ctx rows
_HALF_LOG_2PI = 0.5 * float(np.log(2.0 * np.pi))

# NLL dense tile geometry: TC = 125 * 200 exactly
NP_, NF_ = 125, 200


def _phi_partition(c):
    """Partition in the phi psum bank holding chunk c's (mu, phi1) rows."""
    p, k = c // 4, c % 4
    qq = 8 * ((p // 2) % 2) + 4 * (p % 2) + k
    e = (p // 4) % 4
    return 32 * e + 2 * qq


def _pack_weights(i):
    """Host-side packing of the tiny weights into stationary/bias layouts."""
    hW0, hb0 = i["h_W0"], i["h_b0"]
    hW1, hb1 = i["h_W1"], i["h_b1"]
    hW2, hb2 = i["h_W2"], i["h_b2"]
    gW0, gb0 = i["g_W0"], i["g_b0"]
    gW1, gb1 = i["g_W1"], i["g_b1"]
    gW2, gb2 = i["g_W2"], i["g_b2"]
    gW3, gb3 = i["g_W3"], i["g_b3"]
    gW4, gb4 = i["g_W4"], i["g_b4"]

    W0bd = np.zeros((8, 32), np.float32)
    B0S = np.zeros(128, np.float32)
    for k in range(4):
        W0bd[2 * k:2 * k + 2, 8 * k:8 * k + 8] = hW0
        for j in range(4):
            B0S[32 * j + 8 * k:32 * j + 8 * k + 8] = hb0
    W1S = np.zeros((128, 128), np.float32)
    B1S = np.zeros(128, np.float32)
    for ii in range(4):
        for k in range(4):
            W1S[32 * ii + 8 * k:32 * ii + 8 * k + 8, 32 * k:32 * k + 32] = hW1
    for k in range(4):
        B1S[32 * k:32 * k + 32] = hb1
    W2X4 = np.zeros((128, 128), np.float32)
    for k in range(4):
        W2X4[32 * k:32 * k + 32, :] = hW2
    # padded O rows: constant relu1 activation, folded into the hb2 bias
    a0p = np.maximum(hb0, 0.0)
    a1p = np.maximum(a0p @ hW1 + hb1, 0.0)
    B2C = hb2 - (N_PAD_H / NCTX) * (a1p @ hW2)

    G0X2 = np.zeros((128, 128), np.float32)
    B0X2 = np.zeros(128, np.float32)
    for j in range(2):
        G0X2[:, 64 * j:64 * j + 64] = gW0[:128, :]
        B0X2[64 * j:64 * j + 64] = gb0
    # K=2 paired L0 stationary: rows (32b+j) -> w0l in cols 64j:64j+64
    W0L = np.zeros((128, 128), np.float32)
    for base in range(0, 128, 32):
        for j in range(2):
            W0L[base + j, 64 * j:64 * j + 64] = gW0[128, :]
    G1S = np.zeros((128, 64), np.float32)
    B1G = np.zeros(128, np.float32)
    for j in range(2):
        G1S[64 * j:64 * j + 64, 32 * j:32 * j + 32] = gW1
    for m in range(4):
        B1G[32 * m:32 * m + 32] = gb1
    G2S = np.zeros((128, 64), np.float32)
    B2G = np.zeros(128, np.float32)
    for m in range(4):
        G2S[32 * m:32 * m + 32, 16 * m:16 * m + 16] = gW2
    for mm in range(8):
        B2G[16 * mm:16 * mm + 16] = gb2
    G3S = np.zeros((128, 64), np.float32)
    B3G = np.zeros(128, np.float32)
    for mm in range(8):
        G3S[16 * mm:16 * mm + 16, 8 * mm:8 * mm + 8] = gW3
    for qq in range(16):
        B3G[8 * qq:8 * qq + 8] = gb3
    G4S = np.zeros((128, 32), np.float32)
    B4G = np.zeros(128, np.float32)
    for qq in range(16):
        G4S[8 * qq:8 * qq + 8, 2 * qq:2 * qq + 2] = gW4
    for t in range(64):
        B4G[2 * t:2 * t + 2] = gb4

    w = dict(W0bd=W0bd, B0S=B0S, W1S=W1S, B1S=B1S, W2X4=W2X4, B2C=B2C,
             G0X2=G0X2, B0X2=B0X2, W0L=W0L, G1S=G1S, B1G=B1G, G2S=G2S,
             B2G=B2G, G3S=G3S, B3G=B3G, G4S=G4S, B4G=B4G)
    out = {}
    for k, v in w.items():
        v = np.asarray(v, np.float32)
        if v.ndim == 1:
            v = v[:, None]
        out[k] = np.ascontiguousarray(v)
    return out


def _pack_core_inputs(O, T, c):
    """Per-core activation input layouts (O8, TGH, TYH)."""
    Osh = O[c * OC:(c + 1) * OC]          # [12500, 2]
    Tsh = T[c * TC:(c + 1) * TC]          # [25000, 2]
    # O8[2k+r, 2000P+500j+f] = O row (500*(16P+4j+k)+f), col r
    O8 = np.zeros((8, 2000 * H_PASSES), np.float32)
    for P in range(H_PASSES):
        for j in range(4):
            for k in range(4):
                ch = 16 * P + 4 * j + k
                if ch >= H_CHUNKS:
                    continue
                blk = Osh[ch * F:(ch + 1) * F]    # [500, 2]
                O8[2 * k:2 * k + 2,
                   2000 * P + 500 * j:2000 * P + 500 * (j + 1)] = blk.T
    # TGH[k, 500p+f] = x of chunk 4p+k
    TGH = np.zeros((4, F * G_PASSES), np.float32)
    tx = Tsh[:, 0]
    for p in range(G_PASSES):
        for k in range(4):
            ch = 4 * p + k
            if ch >= G_CHUNKS:
                continue
            TGH[k, F * p:F * (p + 1)] = tx[ch * F:(ch + 1) * F]
    TYH = np.ascontiguousarray(Tsh[:, 1].reshape(NP_, NF_))
    return dict(O8=O8, TGH=TGH, TYH=TYH)


_W_SHAPES = dict(W0bd=(8, 32), B0S=(128, 1), W1S=(128, 128), B1S=(128, 1),
                 W2X4=(128, 128), B2C=(128, 1), G0X2=(128, 128),
                 B0X2=(128, 1), W0L=(128, 128), G1S=(128, 64), B1G=(128, 1),
                 G2S=(128, 64), B2G=(128, 1), G3S=(128, 64), B3G=(128, 1),
                 G4S=(128, 32), B4G=(128, 1))


def _build_kernel():
    nc = bacc.Bacc("TRN2", target_bir_lowering=False, debug=False,
                   num_devices=N_CORES)
    ins = {}
    for name, shp in _W_SHAPES.items():
        ins[name] = nc.dram_tensor(name, list(shp), FP,
                                   kind="ExternalInput").ap()
    ins["O8"] = nc.dram_tensor("O8", [8, 2000 * H_PASSES], FP,
                               kind="ExternalInput").ap()
    ins["TGH"] = nc.dram_tensor("TGH", [4, F * G_PASSES], FP,
                                kind="ExternalInput").ap()
    ins["TYH"] = nc.dram_tensor("TYH", [NP_, NF_], FP,
                                kind="ExternalInput").ap()
    phi_out = nc.dram_tensor("phi_out", [2, TC], FP,
                             kind="ExternalOutput").ap()
    lp_out = nc.dram_tensor("lp_out", [NP_, 1], FP,
                            kind="ExternalOutput").ap()

    with tile.TileContext(nc) as tc:
        _emit(nc, tc, ins, phi_out, lp_out)
    nc.compile()
    return nc


def _emit(nc, tc, ins, phi_out, lp_out):
    dma = nc.sync.dma_start

    with tc.tile_pool(name="consts", bufs=1) as cp, \
         tc.tile_pool(name="persist", bufs=1) as pe, \
         tc.tile_pool(name="dram", bufs=1, space="DRAM") as dp:
        W = {}
        for name, shp in _W_SHAPES.items():
            t = cp.tile(list(shp), FP, tag=name)
            dma(out=t, in_=ins[name])
            W[name] = t
        O8s = cp.tile([8, 2000 * H_PASSES], FP, tag="O8s")
        dma(out=O8s, in_=ins["O8"])

        # ------------- phase H: context net + pooled sum -------------
        s2 = pe.tile([128, 1], FP, tag="s2")
        with tc.tile_pool(name="hsb", bufs=2) as hs, \
             tc.tile_pool(name="hz0", bufs=2, space="PSUM") as pz0, \
             tc.tile_pool(name="hz1", bufs=4, space="PSUM") as pz1:
            s2p = pe.tile([128, H_PASSES * 4], FP, tag="s2p")
            for P in range(H_PASSES):
                z0 = pz0.tile([128, F], FP, tag="z0")
                for j in range(4):
                    nc.tensor.matmul(
                        out=z0[32 * j:32 * j + 32, :],
                        lhsT=W["W0bd"][0:8, :],
                        rhs=O8s[0:8,
                                2000 * P + 500 * j:2000 * P + 500 * (j + 1)],
                        tile_position=(0, 32 * j))
                a0 = hs.tile([128, F], FP, tag="a0")
                nc.scalar.activation(out=a0, in_=z0, func=AF.Relu,
                                     bias=W["B0S"])
                for i in range(4):
                    z1 = pz1.tile([128, F], FP, tag="z1")
                    nc.tensor.matmul(
                        out=z1,
                        lhsT=W["W1S"][32 * i:32 * i + 32, :],
                        rhs=a0[32 * i:32 * i + 32, :],
                        tile_position=(32 * i, 0))
                    a1 = hs.tile([128, F], FP, tag="a1")
                    nc.scalar.activation(
                        out=a1, in_=z1, func=AF.Relu, bias=W["B1S"],
                        accum_out=s2p[:, 4 * P + i:4 * P + i + 1])
            nc.vector.tensor_reduce(out=s2, in_=s2p,
                                    axis=mybir.AxisListType.X, op=ALU.add)

        # ------------- AllReduce of the pooled sum -------------
        cc_in = dp.tile([128, 1], FP)
        cc_out = dp.tile([128, 1], FP)
        dma(out=cc_in, in_=s2)
        nc.gpsimd.collective_compute(
            "AllReduce", ALU.add,
            replica_groups=[list(range(N_CORES))],
            ins=[cc_in.opt()], outs=[cc_out.opt()])
        s2ar = pe.tile([128, 1], FP, tag="s2ar")
        dma(out=s2ar, in_=cc_out)

        # r = (s2/Nctx) @ hW2 + b2'; c0x2 = r @ gW0[:128] + gb0
        s2s = pe.tile([128, 1], FP, tag="s2s")
        nc.vector.tensor_scalar_mul(s2s, s2ar, 1.0 / NCTX)
        c0s = pe.tile([128, 1], FP, tag="c0s")
        with tc.tile_pool(name="rps", bufs=2, space="PSUM") as rps:
            rp = rps.tile([128, 1], FP, tag="rp")
            nc.tensor.matmul(out=rp, lhsT=W["W2X4"], rhs=s2s,
                             tile_position=(0, 0))
            r_sb = pe.tile([128, 1], FP, tag="r_sb")
            nc.vector.tensor_scalar_add(r_sb, rp, W["B2C"])
            c0p = rps.tile([128, 1], FP, tag="c0p")
            nc.tensor.matmul(out=c0p, lhsT=W["G0X2"], rhs=r_sb,
                             tile_position=(0, 0))
            nc.vector.tensor_scalar_add(c0s, c0p, W["B0X2"])

        # ------------- phase G: target net -------------
        phi_buf = dp.tile([2, TC], FP)
        with tc.tile_pool(name="gsb", bufs=2) as gs, \
             tc.tile_pool(name="gzA", bufs=4, space="PSUM") as pzA, \
             tc.tile_pool(name="gzB", bufs=1, space="PSUM") as pzB, \
             tc.tile_pool(name="gzC", bufs=1, space="PSUM") as pzC, \
             tc.tile_pool(name="gzD", bufs=1, space="PSUM") as pzD, \
             tc.tile_pool(name="gzE", bufs=1, space="PSUM") as pzE:
            zE = pzE.tile([128, F], FP, tag="zE")
            zC = zD = None
            for p in range(G_PASSES):
                xg = gs.tile([128, F], FP, tag="xg")
                dma(out=xg[0:2, :], in_=ins["TGH"][0:2, F * p:F * (p + 1)])
                dma(out=xg[32:34, :], in_=ins["TGH"][2:4, F * p:F * (p + 1)])
                # L0: K=2 paired outer products, 2 concurrent row-tiles
                zA = [pzA.tile([128, F], FP, tag="zA", name=f"zA{b_}")
                      for b_ in range(2)]
                for b in range(2):
                    nc.tensor.matmul(
                        out=zA[b],
                        lhsT=W["W0L"][32 * b:32 * b + 2, :],
                        rhs=xg[32 * b:32 * b + 2, :],
                        tile_position=(32 * b, 0))
                aA = []
                for b in range(2):
                    t = gs.tile([128, F], FP, tag=f"aA{b}")
                    nc.scalar.activation(out=t, in_=zA[b], func=AF.Relu,
                                         bias=c0s)
                    aA.append(t)
                # L1: 64->32, two col-tiles into one bank
                zB = pzB.tile([128, F], FP, tag="zB")
                for b in range(2):
                    nc.tensor.matmul(out=zB[64 * b:64 * b + 64, :],
                                     lhsT=W["G1S"], rhs=aA[b],
                                     tile_position=(0, 64 * b))
                aB = gs.tile([128, F], FP, tag="aB")
                nc.vector.tensor_scalar(aB, zB, W["B1G"], 0.0,
                                        op0=ALU.add, op1=ALU.max)
                # L2: 32->16, one col-tile per pass parity
                q = p % 2
                if q == 0:
                    zC = pzC.tile([128, F], FP, tag="zC")
                nc.tensor.matmul(out=zC[64 * q:64 * q + 64, :],
                                 lhsT=W["G2S"], rhs=aB,
                                 tile_position=(0, 64 * q))
                if q == 1:
                    aC = gs.tile([128, F], FP, tag="aC")
                    nc.vector.tensor_scalar(aC, zC, W["B2G"], 0.0,
                                            op0=ALU.add, op1=ALU.max)
                    # L3: 16->8
                    w_ = (p // 2) % 2
                    if w_ == 0:
                        zD = pzD.tile([128, F], FP, tag="zD")
                    nc.tensor.matmul(out=zD[64 * w_:64 * w_ + 64, :],
                                     lhsT=W["G3S"], rhs=aC,
                                     tile_position=(0, 64 * w_))
                    if p % 4 == 3:
                        aD = gs.tile([128, F], FP, tag="aD")
                        nc.vector.tensor_scalar(aD, zD, W["B3G"], 0.0,
                                                op0=ALU.add, op1=ALU.max)
                        # L4: 8->2 into the persistent phi bank
                        e = (p // 4) % 4
                        nc.tensor.matmul(out=zE[32 * e:32 * e + 32, :],
                                         lhsT=W["G4S"], rhs=aD,
                                         tile_position=(0, 32 * e))
            phi_s = pe.tile([128, F], FP, tag="phi_s")
            nc.vector.tensor_scalar_add(phi_s, zE, W["B4G"])
            for c in range(G_CHUNKS):
                pp = _phi_partition(c)
                dma(out=phi_buf[0:2, F * c:F * (c + 1)],
                    in_=phi_s[pp:pp + 2, :])
            dma(out=phi_out, in_=phi_buf)

        # ------------- NLL on dense [125, 200] tiles -------------
        mu_t = pe.tile([NP_, NF_], FP, tag="mu_t")
        ph1_t = pe.tile([NP_, NF_], FP, tag="ph1_t")
        y_t = pe.tile([NP_, NF_], FP, tag="y_t")
        row0 = phi_buf[0:1, :]
        row1 = phi_buf[1:2, :]
        dma(out=mu_t, in_=bass.AP(tensor=row0.tensor, offset=row0.offset,
                                  ap=[[NF_, NP_], [1, NF_]]))
        dma(out=ph1_t, in_=bass.AP(tensor=row1.tensor, offset=row1.offset,
                                   ap=[[NF_, NP_], [1, NF_]]))
        dma(out=y_t, in_=ins["TYH"])
        # sig = softplus(ph1) = ln(1 + exp(ph1)); Exp/Ln share one ACT table
        ex = pe.tile([NP_, NF_], FP, tag="ex")
        nc.scalar.activation(out=ex, in_=ph1_t, func=AF.Exp)
        sig = pe.tile([NP_, NF_], FP, tag="sig")
        nc.scalar.activation(out=sig, in_=ex, func=AF.Ln, bias=1.0)
        lsig = pe.tile([NP_, NF_], FP, tag="lsig")
        lsig_sum = pe.tile([NP_, 1], FP, tag="lsig_sum")
        nc.scalar.activation(out=lsig, in_=sig, func=AF.Ln,
                             accum_out=lsig_sum)
        rs = pe.tile([NP_, NF_], FP, tag="rs")
        nc.vector.reciprocal(out=rs, in_=sig)
        qd = pe.tile([NP_, NF_], FP, tag="qd")
        nc.vector.tensor_sub(qd, y_t, mu_t)
        z = pe.tile([NP_, NF_], FP, tag="z")
        nc.vector.tensor_mul(z, qd, rs)
        sq = pe.tile([NP_, NF_], FP, tag="sq")
        zz_sum = pe.tile([NP_, 1], FP, tag="zz_sum")
        nc.scalar.activation(out=sq, in_=z, func=AF.Square,
                             accum_out=zz_sum)
        lp_p = pe.tile([NP_, 1], FP, tag="lp_p")
        nc.vector.scalar_tensor_tensor(out=lp_p, in0=zz_sum, scalar=-0.5,
                                       in1=lsig_sum, op0=ALU.mult,
                                       op1=ALU.subtract)
        dma(out=lp_out, in_=lp_p)


_NC_CACHE = None


def kernel(**inputs):
    global _NC_CACHE
    if _NC_CACHE is None:
        _NC_CACHE = _build_kernel()
    nc = _NC_CACHE

    w = _pack_weights({k: np.asarray(v, np.float32)
                       for k, v in inputs.items() if k not in ("O", "T")})
    O = np.ascontiguousarray(np.asarray(inputs["O"], np.float32))
    T = np.ascontiguousarray(np.asarray(inputs["T"], np.float32))
    in_maps = []
    for c in range(N_CORES):
        m = dict(w)
        m.update(_pack_core_inputs(O, T, c))
        in_maps.append(m)

    res = run_bass_kernel_spmd(nc, in_maps, list(range(N_CORES))).results

    phi = np.concatenate([res[c]["phi_out"] for c in range(N_CORES)],
                         axis=1).T
    phi = np.ascontiguousarray(phi, np.float32)
    lp_total = sum(float(res[c]["lp_out"].sum(dtype=np.float64))
                   for c in range(N_CORES))
    log_prob = np.float32(lp_total / NTGT - _HALF_LOG_2PI)
    return phi, log_prob
